# revision 6
# baseline (speedup 1.0000x reference)
"""Trainium2 Bass kernel for CenterWoParamMultiCosineNearLoss.

loss = mean_b [ 15 - s_b + x_b + N_b / D_b ]   where, per sample b,
  cos_k = <x_b, c_{label_b, k}>  (k = 0..15 sub-centers of own class)
  s = sum_k cos_k, q = sum_k cos_k^2, x = max_k cos_k
  D = 16 - s,  N = 2*(1-x)^2 - 16 + 2*s - q
(algebraically identical to the reference's term1+term2).

Sharding: samples are sorted by label on the host and split into 8
contiguous shards of 1024 (data-parallel, class-clustered).  All matmul
operands are fp8 e4m3 (validated: final-loss rel err ~1e-6) and use
DoubleRow perf mode (256-deep contraction per instruction).  Each
128-row block multiplies against a G-class sub-window of its core's
center window; the per-block offsets follow a fixed schedule shared by
all 8 cores, computed from the data at build time.  One-hot selection
masks are host-precomputed as [P, NB, G] and broadcast over the 16
sub-centers with a stride-0 AP.  Transfers are issued across both HWDGE
rings in consumption order (the rings share ~300 GB/s of HBM, so order,
not ring count, is what matters); x streams in four 2-block chunks so
the PE tracks the DMA front.  Selection is one DVE multiply per block
(psum x mask, written k-major); class-collapse + row stats run in two
halves so the first half hides under the matmul phase; the loss
epilogue is a single-engine DVE chain (no cross-engine hops).  Each
core emits one [1,1] partial that the host sums.
"""

import os
import sys

import numpy as np

for _p in ("/opt/trn_rl_repo", "/root/.axon_site/_ro/trn_rl_repo"):
    if os.path.isdir(_p) and _p not in sys.path:
        sys.path.append(_p)

import ml_dtypes  # noqa: E402

import concourse.tile as tile  # noqa: E402
from concourse import bacc  # noqa: E402
from concourse import mybir  # noqa: E402
from concourse.bass_utils import run_bass_kernel_spmd  # noqa: E402

P = 128          # SBUF partitions
B = 8192         # batch
D = 1024         # feature dim
C = 90           # classes
K = 16           # sub-centers per class
NCORES = 8
SHARD = B // NCORES          # 1024 samples per core
NB = SHARD // P              # 8 row-blocks per core
KT = D // P                  # 8 contraction tiles
KT2 = KT // 2                # 4 DoubleRow contraction pairs
NCH = NB // 2                # 4 x-chunks of 2 row-blocks
NH = NB // 2                 # blocks per stats half

_F32 = mybir.dt.float32
_F16 = mybir.dt.float16
_F8 = mybir.dt.float8e4

_ADD = mybir.AluOpType.add
_MULT = mybir.AluOpType.mult
_SUB = mybir.AluOpType.subtract
_MAX = mybir.AluOpType.max
_AX = mybir.AxisListType.X
_DR = mybir.MatmulPerfMode.DoubleRow

F8NP = ml_dtypes.float8_e4m3
SQ2 = 1.41421356237309515


def _build_program(wc: int, gc: int, offs: list[int]):
    """One SPMD program for all 8 cores.

    wc   = core window width in columns (16 * classes)
    gc   = per-block window width in columns (16 * G)
    offs = per-block column offsets into the core window (shared schedule)
    """
    g = gc // K
    nc = bacc.Bacc(None, target_bir_lowering=False)
    xch = [
        nc.declare_dram_parameter(f"x{j}", [P, KT, 2 * P], _F8, isOutput=False)
        for j in range(NCH)
    ]
    cw = nc.declare_dram_parameter("cw", [P, KT, wc], _F8, isOutput=False)
    mk = nc.declare_dram_parameter("mk", [P, NB, g], _F16, isOutput=False)
    out = nc.declare_dram_parameter("out", [1, 1], _F32, isOutput=True)

    with tile.TileContext(nc) as tc:
        with (
            tc.tile_pool(name="iop", bufs=1) as iop,
            tc.tile_pool(name="stats", bufs=1) as stats,
            tc.tile_pool(name="pp", bufs=6, space="PSUM") as pp,
            tc.tile_pool(name="ppf", bufs=1, space="PSUM") as ppf,
        ):
            ones = iop.tile([P, 1], _F32)
            nc.vector.memset(ones[:, :], 1.0)

            # --- DMA issues in consumption order, alternating rings ------
            # scalar ring: cw, mk, x1, x3 ; sync ring: x0, x2
            cwt = iop.tile([P, KT, wc], _F8)
            nc.scalar.dma_start(out=cwt[:, :, :], in_=cw[:, :, :])
            xts = [
                iop.tile([P, KT, 2 * P], _F8, name=f"xt{j}", tag=f"x{j}")
                for j in range(NCH)
            ]
            nc.sync.dma_start(out=xts[0][:, :, :], in_=xch[0][:, :, :])
            mkt = iop.tile([P, NB, g], _F16)
            nc.scalar.dma_start(out=mkt[:, :, :], in_=mk[:, :, :])
            nc.scalar.dma_start(out=xts[1][:, :, :], in_=xch[1][:, :, :])
            nc.sync.dma_start(out=xts[2][:, :, :], in_=xch[2][:, :, :])
            nc.scalar.dma_start(out=xts[3][:, :, :], in_=xch[3][:, :, :])

            # selected cos values per block, k-major: [p, block, k, c]
            sel = stats.tile([P, NB, K, g], _F16)
            dsel = stats.tile([P, NB, K], _F16)
            mx = stats.tile([P, NB], _F16)
            ssum = stats.tile([P, NB], _F16)
            qsum = stats.tile([P, NB], _F16)

            def half_stats(ha):
                s0 = ha * NH
                sl = slice(s0, s0 + NH)
                # fp16 partials are safe: |cos|<=1, 16-term sums, loss
                # tolerance 2e-2 (measured end-to-end ~1e-6)
                with nc.allow_low_precision(reason="fp16 stats, loose tolerance"):
                    nc.vector.tensor_reduce(
                        out=dsel[:, sl, :], in_=sel[:, sl, :, :], axis=_AX, op=_ADD
                    )
                    nc.vector.tensor_reduce(
                        out=mx[:, sl], in_=dsel[:, sl, :], axis=_AX, op=_MAX
                    )
                    nc.vector.tensor_reduce(
                        out=ssum[:, sl], in_=dsel[:, sl, :], axis=_AX, op=_ADD
                    )
                    sq = stats.tile([P, NH, K], _F16, tag=f"sq{ha}")
                    nc.vector.tensor_tensor(
                        out=sq[:, :, :], in0=dsel[:, sl, :], in1=dsel[:, sl, :],
                        op=_MULT,
                    )
                    nc.vector.tensor_reduce(
                        out=qsum[:, sl], in_=sq[:, :, :], axis=_AX, op=_ADD
                    )

            for i in range(NB):
                xt, xo = xts[i // 2], (i % 2) * P
                o = offs[i] * K
                ps = pp.tile([P, gc], _F32)
                for t in range(KT2):
                    nc.tensor.matmul(
                        ps[:, :],
                        lhsT=xt[:, 2 * t : 2 * t + 2, xo : xo + P],
                        rhs=cwt[:, 2 * t : 2 * t + 2, o : o + gc],
                        start=(t == 0),
                        stop=(t == KT2 - 1),
                        perf_mode=_DR,
                    )
                # masked select; write transposed (c-major -> k-major); the
                # per-class mask value broadcasts over the 16 sub-centers
                nc.vector.tensor_tensor(
                    out=sel[:, i, :, :].rearrange("p k c -> p c k"),
                    in0=ps[:, :].rearrange("p (c k) -> p c k", k=K),
                    in1=mkt[:, i, :, None].broadcast_to([P, g, K]),
                    op=_MULT,
                )
                if i == NH - 1:
                    half_stats(0)
            half_stats(1)

            # --- epilogue: single serial DVE chain -----------------------
            #   Dd = 16-s ; rs = 1/Dd ; md2 = sqrt2*(1-x) ; u2 = md2^2
            #   t1 = s-q ; t2 = t1-Dd ; nf = u2+t2 ; frac = nf*rs
            #   gg = x-s ; hh = gg+frac ; rowsum = sum(hh)
            sd = stats.tile([P, NB], _F32)
            nc.vector.tensor_scalar(
                out=sd[:, :], in0=ssum[:, :], scalar1=-1.0, scalar2=float(K),
                op0=_MULT, op1=_ADD,
            )
            rs = stats.tile([P, NB], _F32)
            nc.vector.reciprocal(rs[:, :], sd[:, :])
            md2 = stats.tile([P, NB], _F32)
            nc.vector.tensor_scalar(
                out=md2[:, :], in0=mx[:, :], scalar1=-SQ2, scalar2=SQ2,
                op0=_MULT, op1=_ADD,
            )
            u2 = stats.tile([P, NB], _F32)
            nc.vector.tensor_tensor(out=u2[:, :], in0=md2[:, :], in1=md2[:, :], op=_MULT)
            t1 = stats.tile([P, NB], _F32)
            nc.vector.tensor_tensor(out=t1[:, :], in0=ssum[:, :], in1=qsum[:, :], op=_SUB)
            t2 = stats.tile([P, NB], _F32)
            nc.vector.tensor_tensor(out=t2[:, :], in0=t1[:, :], in1=sd[:, :], op=_SUB)
            nf = stats.tile([P, NB], _F32)
            nc.vector.tensor_tensor(out=nf[:, :], in0=u2[:, :], in1=t2[:, :], op=_ADD)
            frac = stats.tile([P, NB], _F32)
            nc.vector.tensor_tensor(out=frac[:, :], in0=nf[:, :], in1=rs[:, :], op=_MULT)
            gg = stats.tile([P, NB], _F32)
            nc.vector.tensor_tensor(out=gg[:, :], in0=mx[:, :], in1=ssum[:, :], op=_SUB)
            hh = stats.tile([P, NB], _F32)
            nc.vector.tensor_tensor(out=hh[:, :], in0=gg[:, :], in1=frac[:, :], op=_ADD)
            rowsum = stats.tile([P, 1], _F32)
            nc.vector.tensor_reduce(out=rowsum[:, :], in_=hh[:, :], axis=_AX, op=_ADD)
            # cross-partition sum via ones-matmul -> single 4B output packet
            psc = ppf.tile([1, 1], _F32)
            nc.tensor.matmul(
                psc[:, :], lhsT=rowsum[:, :], rhs=ones[:, :], start=True, stop=True
            )
            outsb = stats.tile([1, 1], _F32)
            nc.vector.tensor_copy(out=outsb[:, :], in_=psc[:, :])
            nc.sync.dma_start(out=out[:, :], in_=outsb[:, :])

    nc.finalize()
    return nc


def _prep_inputs(x, labels, centers):
    """Host-side sharding/layout prep. Returns (in_maps, wc, gc, offs)."""
    labels = np.asarray(labels).astype(np.int64)
    x = np.ascontiguousarray(np.asarray(x, dtype=np.float32))
    centers = np.asarray(centers, dtype=np.float32)

    perm = np.argsort(labels, kind="stable")
    ls = labels[perm]

    # per-core window start = first class of the shard (no clamping; the
    # center matrix is zero-padded on the right so windows may run past C)
    starts = [int(ls[m * SHARD]) for m in range(NCORES)]
    # fixed per-block offset schedule shared by all cores
    offs, g = [], 0
    for i in range(NB):
        lo, hi = C, -1
        for m in range(NCORES):
            seg = ls[m * SHARD + i * P : m * SHARD + (i + 1) * P]
            lo = min(lo, int(seg[0]) - starts[m])
            hi = max(hi, int(seg[-1]) - starts[m])
        offs.append(lo)
        g = max(g, hi - lo + 1)
    w = max(o + g for o in offs)
    gc, wc = g * K, w * K
    assert gc <= 512, f"block class span {g} too large for a PSUM bank"

    centersT = centers.reshape(C * K, D).T          # [D, C*K] class-major
    pad = max(0, max(starts) + w - C)
    if pad:
        centersT = np.concatenate(
            [centersT, np.zeros((D, pad * K), np.float32)], axis=1
        )

    in_maps = []
    for m in range(NCORES):
        rows = perm[m * SHARD : (m + 1) * SHARD]
        xsT = x[rows].T.astype(F8NP)                               # [D, SHARD]
        xfull = xsT.reshape(KT, P, SHARD).transpose(1, 0, 2)       # [P, KT, SHARD]
        win = centersT[:, K * starts[m] : K * (starts[m] + w)]     # [D, wc]
        cwdev = np.ascontiguousarray(
            win.reshape(KT, P, wc).transpose(1, 0, 2).astype(F8NP)
        )
        lab_local = (ls[m * SHARD : (m + 1) * SHARD]).astype(np.int64)
        lab_pb = lab_local.reshape(NB, P).T - starts[m]            # [P, NB]
        # mask[p, i, c] = 1 if offs[i] + c == local label of row (i*128+p)
        cix = np.asarray(offs)[None, :, None] + np.arange(g)[None, None, :]
        mkdev = np.ascontiguousarray(
            (cix == lab_pb[:, :, None]).astype(np.float16)
        )                                                          # [P, NB, g]
        assert mkdev.sum() == SHARD
        im = {
            f"x{j}": np.ascontiguousarray(xfull[:, :, j * 2 * P : (j + 1) * 2 * P])
            for j in range(NCH)
        }
        im["cw"] = cwdev
        im["mk"] = mkdev
        in_maps.append(im)
    return in_maps, wc, gc, offs


def kernel(x, labels, centers):
    in_maps, wc, gc, offs = _prep_inputs(x, labels, centers)
    nc = _build_program(wc, gc, offs)
    res = run_bass_kernel_spmd(nc, in_maps, core_ids=list(range(NCORES)))
    total = sum(float(r["out"].astype(np.float64).sum()) for r in res.results)
    return np.float32((total + 15.0 * B) / B)
